# revision 14
# baseline (speedup 1.0000x reference)
"""Trainium2 Bass kernel for nn_CausalSelfAttention_77695958385275.

Self-contained: hardcodes shapes/sharding from the problem spec.

Architecture (8 NeuronCores, tensor-parallel over heads, SPMD-homogeneous):
  core c owns: dense head c, sparse head 8+c, full KV head c//2 (for the
  dense head), strided-only KV head 4+c//2 (for the sparse head).
  Every core runs the identical program; only input data differs.

Pipeline per core (all matmuls as float32r views over f32 data):
  1. QKV projection from pre-transposed x^T (dim-major layouts throughout).
  2. RoPE via PE rotate-half matmul + DVE elementwise.
  3. Dense causal attention: S^T tiles -> exp (ACT) -> tri-mask (gpsimd)
     -> ones-matmul row sums (PSUM accum) -> AV accum -> normalize on evict.
  4. Strided sparse attention, same scheme with 46 keys/batch.
  5. AllGather of per-core y^T shards, then a 256-column w_proj shard.
Host: shard/transpose inputs, concatenate output shards.
"""

import math
import ml_dtypes
import numpy as np

import bass_rust
import concourse.bass as bass
import concourse.tile as tile
from concourse import mybir
from concourse.bass_utils import run_bass_kernel_spmd
from concourse.tile import TileContext

# ---------------- problem constants ----------------
B, T, DIM = 2, 2048, 2048
H, KV, HD = 16, 8, 128
NUM_FULL = 8
STRIDE = 45
NS = (T + STRIDE - 1) // STRIDE  # 46 strided keys per batch
SCALE = 1.0 / np.sqrt(np.float32(HD)).astype(np.float32)
N_CORES = 8
BT = B * T  # 4096 tokens total
HALF = HD // 2

F32 = mybir.dt.float32
F32R = mybir.dt.float32r
BF16 = mybir.dt.bfloat16

QCH = 512            # attention q-chunk width
NTCH = T // QCH      # 4 q-chunks per batch
KTILE = 128          # key tile
XCH = 512            # qkv token chunk
CT = DIM // 128      # 16 contraction tiles

ScopedClock = bass_rust.ScopedClock


class SplitDrainTileContext(TileContext):
    """This walrus build allows a single sync-wait slot per CTRL/drain;
    split the tail drain's waits across a chain of single-wait drains."""

    def _drain_and_barrier(self, tick_clock, wait_clock):
        nc = self.nc
        drain_inst = nc.sync.drain()
        wait_clock.add_sem_waits(
            drain_inst.ins, ScopedClock({None: tick_clock.global_clock})
        )
        si = drain_inst.ins.sync_info
        ow = list(si.on_wait or []) if si is not None else []
        if len(ow) > 1:
            si.on_wait = [ow[0]]
            drain_inst.ins.sync_info = si
            for w in ow[1:]:
                d2 = nc.sync.drain()
                s2 = d2.ins.sync_info
                if s2 is None:
                    s2 = bass_rust.SyncInfo(on_wait=[w], on_update=[])
                else:
                    s2.on_wait = [w]
                d2.ins.sync_info = s2
        nc.all_engine_barrier()
        assert self.sems is not None
        popped = nc._tile_sem_poison_stack.pop()
        assert popped is self._sem_poison
        nc.clear_and_free_semaphores(list(self.sems.allocated().values()))
        nc.all_engine_barrier()


def split_multi_waits(nc, max_waits=1):
    """Walrus here rejects >1 sync wait on several instruction formats; move
    extra waits onto preceding same-engine NoOps."""
    for f in nc.m.functions:
        for b in f.blocks:
            new = []
            changed = False
            for inst in b.instructions:
                si = inst.sync_info
                ow = list(si.on_wait) if (si is not None and si.on_wait) else []
                if len(ow) > max_waits:
                    changed = True
                    for w in ow[:-max_waits]:
                        nop = mybir.InstNoOp(
                            name=nc.get_next_instruction_name(), ins=[], outs=[]
                        )
                        nop.engine = inst.engine
                        nop.sync_info = bass_rust.SyncInfo(on_wait=[w], on_update=[])
                        new.append(nop)
                    si.on_wait = ow[-max_waits:]
                    inst.sync_info = si
                new.append(inst)
            if changed:
                b.instructions = new


# ---------------- host-side constant tables ----------------

def _rope_tables():
    pos = np.arange(T, dtype=np.float32)
    freqs = (np.arange(HALF, dtype=np.float32) / np.float32(HALF)).astype(np.float32)
    ang = pos[:, None] * freqs[None, :]          # [T, 64] f32
    cosv = np.cos(ang.astype(np.float64)).astype(np.float32).T   # [64, T]
    sinv = np.sin(ang.astype(np.float64)).astype(np.float32).T
    cc = np.concatenate([cosv, cosv], axis=0)    # [128, T]
    ss = np.concatenate([sinv, sinv], axis=0)
    ccT = np.concatenate([cc, cc], axis=1)       # [128, 4096] (b0|b1)
    ssT = np.concatenate([ss, ss], axis=1)
    sp = np.arange(0, T, STRIDE)
    ccS = np.concatenate([cc[:, sp], cc[:, sp]], axis=1)  # [128, 92]
    ssS = np.concatenate([ss[:, sp], ss[:, sp]], axis=1)
    return (np.ascontiguousarray(ccT), np.ascontiguousarray(ssT),
            np.ascontiguousarray(ccS), np.ascontiguousarray(ssS))


def _const_tables():
    ccT, ssT, ccS, ssS = _rope_tables()
    mrotT = np.zeros((HD, HD), np.float32)
    for i in range(HALF):
        mrotT[i + HALF, i] = -1.0   # (M^T)[i+64, i]: out[0:64] = -q[64:128]
        mrotT[i, i + HALF] = 1.0    # out[64:128] = +q[0:64]
    ident = np.eye(128, dtype=np.float32)
    ones = np.ones((128, 128), np.float32)
    # additive causal masks: 0 where valid, -1e9 where masked (added to
    # scores in PSUM via an identity-lhsT matmul; exp then yields 0)
    tri = np.where(np.arange(128)[None, :] >= np.arange(128)[:, None],
                   0.0, -1e9).astype(np.float32)          # [jk, x]
    q = np.arange(T)
    smask = np.where(q[None, :] >= (STRIDE * np.arange(NS))[:, None],
                     0.0, -1e9).astype(np.float32)        # [46, T]
    return ccT, ssT, ccS, ssS, mrotT, ident, ones, tri, smask


# ---------------- device program ----------------

def build_program():
    nc = bass.Bass(num_devices=N_CORES)

    xT = nc.dram_tensor("xT", [DIM, BT], BF16, kind="ExternalInput")
    xsT = nc.dram_tensor("xsT", [DIM, B * NS], BF16, kind="ExternalInput")
    wqT = nc.dram_tensor("wqT", [DIM, 2 * HD], BF16, kind="ExternalInput")
    wkT = nc.dram_tensor("wkT", [DIM, HD], BF16, kind="ExternalInput")
    wvT = nc.dram_tensor("wvT", [DIM, HD], BF16, kind="ExternalInput")
    wksT = nc.dram_tensor("wksT", [DIM, HD], BF16, kind="ExternalInput")
    wvsT = nc.dram_tensor("wvsT", [DIM, HD], BF16, kind="ExternalInput")
    wpT = nc.dram_tensor("wpT", [DIM, 2 * HD], BF16, kind="ExternalInput")
    outT = nc.dram_tensor("outT", [2 * HD, BT], F32, kind="ExternalOutput")

    HT = T // 2  # tokens per AllGather piece
    agin = [[nc.dram_tensor(f"agin{b}_{h}", [2 * HD, HT], BF16, kind="Internal")
             for h in range(2)] for b in range(B)]
    agout = [[nc.dram_tensor(f"agout{b}_{h}", [N_CORES * 2 * HD, HT], BF16,
                             kind="Internal", addr_space="Shared")
              for h in range(2)] for b in range(B)]

    wu_in = nc.dram_tensor("wu_in", [8, 64], BF16, kind="Internal")
    wu_out = nc.dram_tensor("wu_out", [64, 64], BF16, kind="Internal",
                            addr_space="Shared")

    ccT_h, ssT_h, ccS_h, ssS_h, mrotT_h, ident_h, ones_h, tri_h, smask_h = \
        _const_tables()
    ccT_d = nc.inline_tensor(ccT_h, "ccT")
    ssT_d = nc.inline_tensor(ssT_h, "ssT")
    ccS_d = nc.inline_tensor(ccS_h, "ccS")
    ssS_d = nc.inline_tensor(ssS_h, "ssS")
    mrotT_d = nc.inline_tensor(mrotT_h, "mrotT")
    ident_d = nc.inline_tensor(ident_h, "ident")
    ones_d = nc.inline_tensor(ones_h, "onesm")
    tri_d = nc.inline_tensor(tri_h, "trim")
    smask_d = nc.inline_tensor(smask_h, "smask")

    AF = mybir.ActivationFunctionType
    OP = mybir.AluOpType

    with SplitDrainTileContext(nc) as tc:
        with tc.tile_pool(name="persist", bufs=1) as PP:
            # persistent SBUF state
            qdT = PP.tile([128, BT], F32R, tag="qdT")    # dense-head q^T (roped in place)
            qsT = PP.tile([128, BT], F32R, tag="qsT")    # sparse-head q^T
            kT = PP.tile([128, BT], F32R, tag="kT")      # full k^T of kv_a
            vtok = PP.tile([128, BT], F32R, tag="vtok")  # v token-major, 32 tiles of [128t,128d]
            ksT = PP.tile([128, B * NS], F32R, tag="ksT")     # strided k^T of kv_b
            vs = PP.tile([NS, B * HD], F32R, tag="vs")        # strided v token-major [46, (b,d)]
            mrot = PP.tile([128, 128], F32R, tag="mrot")
            ident = PP.tile([128, 128], F32, tag="ident")
            identr = PP.tile([128, 128], F32R, tag="identr")
            ones = PP.tile([128, 128], F32R, tag="ones")
            tri = PP.tile([128, 128], F32R, tag="tri")
            smask = PP.tile([NS, T], F32R, tag="smask")
            ccS = PP.tile([128, B * NS], F32, tag="ccS")
            ssS = PP.tile([128, B * NS], F32, tag="ssS")

            nc.sync.dma_start(mrot[:], mrotT_d[:].bitcast(F32R))
            nc.sync.dma_start(ident[:], ident_d[:])
            nc.sync.dma_start(identr[:], ident_d[:].bitcast(F32R))
            nc.sync.dma_start(ones[:], ones_d[:].bitcast(F32R))
            nc.sync.dma_start(tri[:], tri_d[:].bitcast(F32R))
            nc.sync.dma_start(smask[:], smask_d[:].bitcast(F32R))
            nc.sync.dma_start(ccS[:], ccS_d[:])
            nc.sync.dma_start(ssS[:], ssS_d[:])

            # tiny warmup AllGather: absorbs collective cold-start + rank skew
            # long before the real gathers
            nc.gpsimd.collective_compute(
                "AllGather", OP.bypass,
                ins=[wu_in[:]], outs=[wu_out[:]],
                replica_groups=[list(range(N_CORES))],
            )

            # ------- Phase 1a: strided k/v for the sparse kv head (startup fill)
            with tc.tile_pool(name="wstr", bufs=1) as WS, \
                 tc.tile_pool(name="sps", bufs=2, space="PSUM") as SPS, \
                 tc.tile_pool(name="rtmp0", bufs=2) as RT0:
                xs_sb = WS.tile([128, CT, B * NS], BF16, tag="xs")
                wks_sb = WS.tile([128, CT, HD], BF16, tag="wks")
                wvs_sb = WS.tile([128, CT, HD], BF16, tag="wvs")
                nc.sync.dma_start(xs_sb[:], xsT[:].rearrange("(a p) n -> p a n", p=128))
                nc.sync.dma_start(wks_sb[:], wksT[:].rearrange("(a p) n -> p a n", p=128))
                nc.sync.dma_start(wvs_sb[:], wvsT[:].rearrange("(a p) n -> p a n", p=128))
                ps = SPS.tile([128, B * NS], F32, tag="ks")
                for ci in range(CT):
                    nc.tensor.matmul(ps[:], wks_sb[:, ci, :], xs_sb[:, ci, :],
                                     start=(ci == 0), stop=(ci == CT - 1))
                nc.scalar.copy(ksT[:], ps[:])
                for b in range(B):
                    psv = SPS.tile([NS, HD], F32, tag="vsp")
                    for ci in range(CT):
                        nc.tensor.matmul(
                            psv[:], xs_sb[:, ci, b * NS:(b + 1) * NS],
                            wvs_sb[:, ci, :],
                            start=(ci == 0), stop=(ci == CT - 1))
                    nc.vector.tensor_copy(vs[:, b * HD:(b + 1) * HD], psv[:])
                # strided k rope
                rsw = SPS.tile([128, B * NS], F32, tag="ks")
                nc.tensor.matmul(rsw[:], mrot[:], ksT[:], start=True, stop=True)
                t1 = RT0.tile([128, B * NS], F32, tag="t1s")
                nc.gpsimd.tensor_mul(t1[:], ksT[:], ccS[:])
                t2 = RT0.tile([128, B * NS], F32, tag="t2s")
                nc.vector.scalar_tensor_tensor(
                    t2[:], rsw[:], 1.0, ssS[:], op0=OP.mult, op1=OP.mult)
                nc.vector.tensor_add(ksT[:], t1[:], t2[:])

            # ------- Phase 1b: QKV with per-chunk fused RoPE -------
            with tc.tile_pool(name="wq", bufs=1) as WQ, \
                 tc.tile_pool(name="xs", bufs=2) as XS, \
                 tc.tile_pool(name="rtmp", bufs=3) as RT, \
                 tc.tile_pool(name="vtmp", bufs=2) as VT, \
                 tc.tile_pool(name="qkps", bufs=3, space="PSUM") as QPS, \
                 tc.tile_pool(name="trps", bufs=2, space="PSUM") as TPS, \
                 tc.tile_pool(name="rps", bufs=2, space="PSUM") as RPS:
                wq_sb = WQ.tile([128, CT, 2 * HD], BF16, tag="wq")
                wk_sb = WQ.tile([128, CT, HD], BF16, tag="wk")
                wv_sb = WQ.tile([128, CT, HD], BF16, tag="wv")
                cc = WQ.tile([128, BT], F32, tag="cc")
                ss = WQ.tile([128, BT], F32, tag="ss")
                nc.sync.dma_start(wq_sb[:], wqT[:].rearrange("(a p) n -> p a n", p=128))
                nc.sync.dma_start(wk_sb[:], wkT[:].rearrange("(a p) n -> p a n", p=128))
                nc.sync.dma_start(wv_sb[:], wvT[:].rearrange("(a p) n -> p a n", p=128))
                for tch in range(BT // XCH):
                    c0 = tch * XCH
                    sl = slice(c0, c0 + XCH)
                    x_sb = XS.tile([128, CT, XCH], BF16, tag="x")
                    nc.sync.dma_start(
                        x_sb[:], xT[:, sl].rearrange("(a p) n -> p a n", p=128))
                    nc.sync.dma_start(cc[:, sl], ccT_d[:, sl])
                    nc.sync.dma_start(ss[:, sl], ssT_d[:, sl])
                    # q0, q1, k -> evict -> rope in place
                    for mi, (wt, msl, dst) in enumerate((
                            (wq_sb, slice(0, 128), qdT),
                            (wq_sb, slice(128, 256), qsT),
                            (wk_sb, slice(0, 128), kT))):
                        ps = QPS.tile([128, XCH], F32, tag="mm")
                        for ci in range(CT):
                            nc.tensor.matmul(
                                ps[:], wt[:, ci, msl], x_sb[:, ci, :],
                                start=(ci == 0), stop=(ci == CT - 1))
                        nc.scalar.copy(dst[:, sl], ps[:])
                        rsw = RPS.tile([128, XCH], F32, tag="rsw")
                        nc.tensor.matmul(rsw[:], mrot[:], dst[:, sl],
                                         start=True, stop=True)
                        t1 = RT.tile([128, XCH], F32, tag="t1")
                        nc.gpsimd.tensor_mul(t1[:], dst[:, sl], cc[:, sl])
                        t2 = RT.tile([128, XCH], F32, tag="t2")
                        nc.vector.scalar_tensor_tensor(
                            t2[:], rsw[:], 1.0, ss[:, sl],
                            op0=OP.mult, op1=OP.mult)
                        nc.vector.tensor_add(dst[:, sl], t1[:], t2[:])
                    # v^T then transpose to token-major
                    ps = QPS.tile([128, XCH], F32, tag="mm")
                    for ci in range(CT):
                        nc.tensor.matmul(
                            ps[:], wv_sb[:, ci, :], x_sb[:, ci, :],
                            start=(ci == 0), stop=(ci == CT - 1))
                    vt_sb = VT.tile([128, XCH], F32, tag="vt")
                    nc.scalar.copy(vt_sb[:], ps[:])
                    for sub in range(XCH // 128):
                        pt = TPS.tile([128, 128], F32, tag="tr")
                        nc.tensor.transpose(
                            pt[:], vt_sb[:, sub * 128:(sub + 1) * 128], ident[:])
                        j = (c0 // 128) + sub
                        nc.vector.tensor_copy(vtok[:, j * 128:(j + 1) * 128], pt[:])

            # -------- Phases 3-6: attention -> per-batch AllGather -> proj ----
            with tc.tile_pool(name="pp", bufs=4) as PPOOL, \
                 tc.tile_pool(name="rr", bufs=3) as RR, \
                 tc.tile_pool(name="yev", bufs=3) as YEV, \
                 tc.tile_pool(name="wp", bufs=1) as WPP, \
                 tc.tile_pool(name="ya", bufs=3) as YA, \
                 tc.tile_pool(name="oev", bufs=3) as OEV, \
                 tc.tile_pool(name="sS", bufs=3, space="PSUM") as PS_S, \
                 tc.tile_pool(name="sAcc", bufs=2, space="PSUM") as PS_A, \
                 tc.tile_pool(name="sY", bufs=3, space="PSUM") as PS_Y:
                wp_sb = WPP.tile([128, CT, 2 * HD], BF16, tag="wp")
                nc.sync.dma_start(wp_sb[:], wpT[:].rearrange("(a p) n -> p a n", p=128))

                def proj_piece(b, h):
                    for tch in range(HT // 512):
                        c0 = tch * 512
                        ya = YA.tile([128, CT, 512], BF16, tag="ya")
                        nc.sync.dma_start(
                            ya[:], agout[b][h][:, c0:c0 + 512].rearrange(
                                "(a p) n -> p a n", p=128))
                        oc = b * T + h * HT + c0
                        for o in range(2):
                            ps = PS_S.tile([128, 512], F32, tag="S")
                            for ci in range(CT):
                                nc.tensor.matmul(
                                    ps[:], wp_sb[:, ci, o * 128:(o + 1) * 128],
                                    ya[:, ci, :],
                                    start=(ci == 0), stop=(ci == CT - 1))
                            oe = OEV.tile([128, 512], F32, tag="oe")
                            nc.scalar.copy(oe[:], ps[:])
                            nc.sync.dma_start(
                                outT[o * 128:(o + 1) * 128, oc:oc + 512], oe[:])

                for b in range(B):
                    for J in range(NTCH):
                        qsl = slice(b * T + J * QCH, b * T + (J + 1) * QCH)
                        osl = slice(J * QCH, (J + 1) * QCH)
                        sums = PS_A.tile([128, QCH], F32, tag="sums")
                        yacc = PS_Y.tile([128, QCH], F32, tag="yacc")
                        ntk = (J + 1) * (QCH // KTILE)
                        for i in range(ntk):
                            S = PS_S.tile([128, QCH], F32, tag="S")
                            nc.tensor.matmul(
                                S[:], kT[:, b * T + i * KTILE: b * T + (i + 1) * KTILE],
                                qdT[:, qsl], start=True, stop=True)
                            c0 = max(0, KTILE * i - QCH * J)
                            if c0 + 128 <= QCH and KTILE * i >= QCH * J:
                                nc.tensor.matmul(
                                    S[:, c0:c0 + 128], identr[:], tri[:],
                                    start=False, stop=True, skip_group_check=True)
                            P = PPOOL.tile([128, QCH], F32R, tag="P")
                            nc.scalar.activation(P[:, c0:QCH], S[:, c0:QCH], AF.Exp)
                            nc.tensor.matmul(
                                sums[:, c0:QCH], ones[:], P[:, c0:QCH],
                                start=(i == 0), stop=(i == ntk - 1),
                                skip_group_check=True)
                            j = (b * T) // 128 + i
                            nc.tensor.matmul(
                                yacc[:, c0:QCH], vtok[:, j * 128:(j + 1) * 128],
                                P[:, c0:QCH],
                                start=(i == 0), stop=(i == ntk - 1),
                                skip_group_check=True)
                        rs = RR.tile([128, QCH], F32, tag="rs")
                        nc.vector.reciprocal(rs[:], sums[:])
                        yev = YEV.tile([128, QCH], BF16, tag="ye")
                        nc.vector.scalar_tensor_tensor(
                            yev[:], yacc[:], 1.0, rs[:], op0=OP.mult, op1=OP.mult)
                        nc.sync.dma_start(
                            agin[b][J // 2][0:128, (J % 2) * QCH:(J % 2 + 1) * QCH],
                            yev[:])

                        # ---- sparse head, same (b, J) chunk ----
                        Ssp = PS_S.tile([NS, QCH], F32, tag="S")
                        nc.tensor.matmul(
                            Ssp[:], ksT[:, b * NS:(b + 1) * NS], qsT[:, qsl],
                            start=True, stop=True)
                        nc.tensor.matmul(
                            Ssp[:], identr[0:NS, 0:NS],
                            smask[:, J * QCH:(J + 1) * QCH],
                            start=False, stop=True, skip_group_check=True)
                        Psp = PPOOL.tile([NS, QCH], F32R, tag="P")
                        nc.scalar.activation(Psp[:], Ssp[:], AF.Exp)
                        sums2 = PS_A.tile([128, QCH], F32, tag="sums")
                        nc.tensor.matmul(sums2[:], ones[0:NS, :], Psp[:],
                                         start=True, stop=True)
                        yacc2 = PS_Y.tile([128, QCH], F32, tag="yacc")
                        nc.tensor.matmul(
                            yacc2[:], vs[:, b * HD:(b + 1) * HD], Psp[:],
                            start=True, stop=True)
                        rs2 = RR.tile([128, QCH], F32, tag="rs")
                        nc.vector.reciprocal(rs2[:], sums2[:])
                        yev2 = YEV.tile([128, QCH], BF16, tag="ye")
                        nc.vector.scalar_tensor_tensor(
                            yev2[:], yacc2[:], 1.0, rs2[:], op0=OP.mult, op1=OP.mult)
                        nc.sync.dma_start(
                            agin[b][J // 2][128:256, (J % 2) * QCH:(J % 2 + 1) * QCH],
                            yev2[:])
                        if J % 2 == 1:
                            nc.gpsimd.collective_compute(
                                "AllGather", OP.bypass,
                                ins=[agin[b][J // 2][:]], outs=[agout[b][J // 2][:]],
                                replica_groups=[list(range(N_CORES))],
                            )

                for b in range(B):
                    for h in range(2):
                        proj_piece(b, h)

    split_multi_waits(nc)
    return nc


_PROG_CACHE = {}


def _get_program():
    if "nc" not in _PROG_CACHE:
        _PROG_CACHE["nc"] = build_program()
    return _PROG_CACHE["nc"]


def _host_prep(x, w_attn, w_proj, q_gain, attn_temp):
    x = np.asarray(x, np.float32)
    w_attn = np.asarray(w_attn, np.float32)
    w_proj = np.asarray(w_proj, np.float32)
    q_gain = np.asarray(q_gain, np.float32)
    attn_temp = np.asarray(attn_temp, np.float32)

    BF = ml_dtypes.bfloat16
    xT = np.ascontiguousarray(x.reshape(BT, DIM).T.astype(BF))       # [DIM, BT]
    xs = x[:, ::STRIDE, :]                                           # [B, 46, DIM]
    xsT = np.ascontiguousarray(xs.reshape(B * NS, DIM).T.astype(BF)) # [DIM, 92]

    g = (q_gain * attn_temp * SCALE).astype(np.float32)              # [H]
    wq = w_attn[:H * HD].reshape(H, HD, DIM)
    wq = wq * g[:, None, None]
    wk = w_attn[H * HD:(H + KV) * HD].reshape(KV, HD, DIM)
    wv = w_attn[(H + KV) * HD:].reshape(KV, HD, DIM)

    # w_proj^T with input dims permuted to AG row order:
    # rank r contributes [dense head r | sparse head 8+r]
    perm = np.concatenate(
        [np.concatenate([np.arange(r * HD, (r + 1) * HD),
                         np.arange((8 + r) * HD, (9 + r) * HD)])
         for r in range(N_CORES)])
    wpT_perm = np.ascontiguousarray(w_proj.T[perm, :])               # [DIM, DIM]

    in_maps = []
    for c in range(N_CORES):
        kva, kvb = c // 2, 4 + c // 2
        in_maps.append({
            "xT": xT,
            "xsT": xsT,
            "wqT": np.ascontiguousarray(
                np.concatenate([wq[c], wq[8 + c]], axis=0).T.astype(BF)),
            "wkT": np.ascontiguousarray(wk[kva].T.astype(BF)),
            "wvT": np.ascontiguousarray(wv[kva].T.astype(BF)),
            "wksT": np.ascontiguousarray(wk[kvb].T.astype(BF)),
            "wvsT": np.ascontiguousarray(wv[kvb].T.astype(BF)),
            "wpT": np.ascontiguousarray(
                wpT_perm[:, c * 2 * HD:(c + 1) * 2 * HD].astype(BF)),
        })
    return in_maps


def run(x, w_attn, w_proj, q_gain, attn_temp, trace=False):
    nc = _get_program()
    in_maps = _host_prep(x, w_attn, w_proj, q_gain, attn_temp)
    res = run_bass_kernel_spmd(nc, in_maps, core_ids=list(range(N_CORES)),
                               trace=trace)
    shards = [res.results[c]["outT"] for c in range(N_CORES)]        # [256, BT] each
    outT = np.concatenate(shards, axis=0)                            # [DIM, BT]
    out = outT.T.reshape(B, T, DIM).astype(np.float32)
    return out, res


def kernel(x, w_attn, w_proj, q_gain, attn_temp):
    out, _ = run(x, w_attn, w_proj, q_gain, attn_temp, trace=False)
    return out
